# revision 12
# baseline (speedup 1.0000x reference)
"""Differentiable random-forest layer (inference path) on 8 Trainium2 cores.

Computation (per reference):
    d     = sigmoid(einsum('bf,tfn->btn', x, W))        # [B, T, 255]
    route = prod_l where(IS_LEFT, d[..n..], 1-d[..n..]) # [B, T, 256]
    out   = clip(einsum('btl,tlc->bc', route, P) / T, 0, 1)

Shapes: B=4096, F=1024, T=10 trees, 255 nodes / 256 leaves, C=1000.
Sharding: data-parallel over batch; 512 rows/core; no collectives.

v3: both matmuls in fp8(e4m3) with perf_mode=DoubleRow (256-row contraction
per matmul; measured 216ns per 512-col DR matmul at full p-state = 2x bf16).
The PE p-state drops ~2x after any idle gap and takes ~3us to recover, so
the schedule keeps the PE gapless from warmup to the last matmul:
  - mm1 groups emitted along (b+j) anti-diagonals: starts as soon as W j0
    lands while later W blocks stream in, and each batch chunk's sigmoid/
    routing chain starts as early as the DMA bandwidth allows.
  - routing runs split across DVE (most trees) and GpSimd (rest) per chunk;
    the bf16->fp8 casts of the transposed route run on GpSimd; sigmoids and
    the output descale-copies run on ACT. Chains pipeline under the PE.

Accuracy design (gate: rel err < 2e-2; measured 1.10e-2 for v2):
  - mm1: x fp8, W x64 in fp8; 1/64 folded into the sigmoid input scale.
  - routing in bf16 with the complement trick hi = cur - cur*d (no second
    sigmoid pass); route carries a x256 scale introduced at layer 0.
  - mm2: route fp8 after transpose; P decomposed as P = Ptilde + leafmean:
    the device matmul uses zero-mean Ptilde (x2^17, error-feedback
    quantized along leaves); the exact bias sum_t leafmean/T is added on
    the host. This kills the coherent route-error x mean(P) coupling.
  - reference clip(0,1): upper bound provably inactive; host clips.
"""

from contextlib import ExitStack

import numpy as np
import ml_dtypes

import concourse.bass as bass
import concourse.bacc as bacc
import concourse.mybir as mybir
import concourse.tile as tile
from concourse.tile import add_dep_helper
from concourse.bass_utils import run_bass_kernel_spmd

N_CORES = 8
B, F, T, NODES, LEAFS, C = 4096, 1024, 10, 255, 256, 1000
B_LOC = B // N_CORES            # 512 batch rows per core
BCH = B_LOC // 128              # 4 batch chunks of 128
KC = F // 256                   # 4 DoubleRow contraction chunks for mm1
TP = T // 2                     # 5 tree-pairs (2 trees -> 512 psum cols)
NP = 256                        # padded nodes per tree (255 + 1 pad col)
N_LAYERS = 8
CP = 1024                       # padded classes in SBUF

SP = float(2.0 ** 17)           # Ptilde fp8 scale
SR = 256.0                      # route scale (introduced at routing layer 0)
ALPHA = 1.0 / (SR * SP * T)     # psum2 -> output scale

WARM_N = 40                     # PE warmup matmuls (p-state ramp, ends ~when W j0 lands)

BF16 = mybir.dt.bfloat16
F8 = mybir.dt.float8e4
F32 = mybir.dt.float32
DR = mybir.MatmulPerfMode.DoubleRow
Sigmoid = mybir.ActivationFunctionType.Sigmoid
MULT = mybir.AluOpType.mult
ADD = mybir.AluOpType.add


def _bitrev(x: int, bits: int) -> int:
    r = 0
    for _ in range(bits):
        r = (r << 1) | (x & 1)
        x >>= 1
    return r


# Node-axis permutation: d'[.., off+q] = d[.., off+bitrev_l(q)] per layer l
NODE_PERM = np.empty(NODES, dtype=np.int64)
for _l in range(N_LAYERS):
    _off = (1 << _l) - 1
    for _q in range(1 << _l):
        NODE_PERM[_off + _q] = _off + _bitrev(_q, _l)
# Leaf-axis permutation: P'[t, q, :] = P[t, bitrev_8(q), :]
LEAF_PERM = np.array([_bitrev(q, N_LAYERS) for q in range(LEAFS)], dtype=np.int64)


def build_program() -> bass.Bass:
    nc = bacc.Bacc()

    xT = nc.dram_tensor("xT", [KC, 128, 2, B_LOC], F8, kind="ExternalInput")
    # W j-major: per tree-pair j: [128ki, KC, 2ko, 2*NP]
    w = nc.dram_tensor("w", [TP, KC, 128, 2, 2 * NP], F8, kind="ExternalInput")
    p = nc.dram_tensor("p", [128, T, 2, CP], F8, kind="ExternalInput")
    out = nc.dram_tensor("out", [B_LOC, C], F32, kind="ExternalOutput")

    with tile.TileContext(nc) as tc, ExitStack() as ctx:
        resident = ctx.enter_context(tc.tile_pool(name="resident", bufs=1))
        x_all = resident.tile([128, KC, 2, B_LOC], F8, tag="x_all", name="x_all")
        w_all = resident.tile([128, TP, KC, 2, 2 * NP], F8, tag="w_all", name="w_all")
        p_all = resident.tile([128, T, 2, CP], F8, tag="p_all", name="p_all")
        # Load order: x (every mm1 needs it), W j0 split fine so the first
        # matmul group can start earliest, the rest of W per (j, kc), then P
        # (mm2-only, arrives in time; keeping it last leaves the full HBM
        # bandwidth to the W stream that gates mm1).
        for kc in range(KC):
            nc.sync.dma_start(x_all[:, kc, :, :], xT[kc])
        for kc in range(KC):
            for ko in range(2):
                nc.sync.dma_start(w_all[:, 0, kc, ko, :], w[0, kc][:, ko])
        for j in range(1, TP):
            for kc in range(KC):
                nc.sync.dma_start(w_all[:, j, kc, :, :], w[j, kc])
        for t_ in range(0, T, 2):
            nc.sync.dma_start(p_all[:, t_ : t_ + 2, :, :], p[:, t_ : t_ + 2, :, :])

        dpool = ctx.enter_context(tc.tile_pool(name="dps", bufs=1, space="PSUM"))
        opool = ctx.enter_context(tc.tile_pool(name="ops", bufs=3, space="PSUM"))
        work = ctx.enter_context(tc.tile_pool(name="work", bufs=2))

        # PE warmup (p-state ramp) overlapping the input DMAs; also a dummy
        # activation so ACT's sigmoid table loads before the first real one.
        warm_in = work.tile([128, 128], BF16, tag="warm", name="warm_in", bufs=1)
        nc.vector.memset(warm_in[:, :], 0.0)
        warm_sig = work.tile([128, 1], BF16, tag="wsig", name="warm_sig", bufs=1)
        nc.scalar.activation(warm_sig[:, :], warm_in[:, 0:1], Sigmoid)
        warm_ps = opool.tile([128, 128], F32, tag="warm", name="warm_ps", bufs=1)
        for _ in range(WARM_N):
            nc.tensor.matmul(warm_ps[:, :], warm_in[:, :], warm_in[:, :])

        # d tiles: one per batch chunk, [128, tree, node(padded)] bf16
        ddb = [
            work.tile([128, T, NP], BF16, tag=f"ddb{b}", name=f"ddb{b}", bufs=1)
            for b in range(BCH)
        ]

        def emit_mm1(bi, j):
            # DoubleRow mm1 for (batch chunk bi, tree pair j) + sigmoid
            dps = dpool.tile([128, 2, NP], F32, tag="dps", name="dps", bufs=3)
            for kc in range(KC):
                nc.tensor.matmul(
                    dps[:, :, :],
                    x_all[:, kc, :, bass.ts(bi, 128)],
                    w_all[:, j, kc, :, :],
                    start=(kc == 0),
                    stop=(kc == KC - 1),
                    perf_mode=DR,
                )
            # d = sigmoid(logits/64) (W was scaled x64 on host)
            nc.scalar.activation(
                ddb[bi][:, 2 * j : 2 * j + 2, :], dps[:, :, :], Sigmoid, scale=1.0 / 64
            )

        def emit_routing(bi):
            # bf16 routing on DVE, complement trick, x256 scale.
            d = ddb[bi]
            Ra = work.tile([128, T, 128], BF16, tag="Ra", name="Ra")
            Rb = work.tile([128, T, 128], BF16, tag="Rb", name="Rb")
            routeC = work.tile([128, 2, T, 128], BF16, tag="routeC", name="routeC")
            # layer 0: lo = 256*d0 ; hi = 256 - lo
            first = nc.vector.tensor_scalar_mul(Ra[:, :, 0:1], d[:, :, 0:1], SR)
            nc.vector.tensor_scalar(Ra[:, :, 1:2], d[:, :, 0:1], -SR, SR, MULT, ADD)
            cur, nxt = Ra, Rb
            for l in range(1, N_LAYERS):
                w_l = 1 << l          # prefixes at layer l
                off = w_l - 1         # first node index of layer l
                if l < N_LAYERS - 1:
                    lo, hi = nxt[:, :, 0:w_l], nxt[:, :, w_l : 2 * w_l]
                else:
                    lo, hi = routeC[:, 0, :, :], routeC[:, 1, :, :]
                nc.vector.tensor_mul(lo, cur[:, :, 0:w_l], d[:, :, off : off + w_l])
                nc.vector.tensor_sub(hi, cur[:, :, 0:w_l], lo)
                cur, nxt = nxt, cur
            # transpose [b, leaf] -> [leaf, b] per leaf chunk (bf16 XBAR).
            # Issued on the ACT engine's HWDGE ring so they never queue
            # behind the big W/P input loads on the SP ring.
            rTb = work.tile([128, 2, T, 128], BF16, tag="rTb", name="rTb", bufs=2)
            nc.scalar.dma_start_transpose(rTb[:, 0, :, :], routeC[:, 0])
            nc.scalar.dma_start_transpose(rTb[:, 1, :, :], routeC[:, 1])
            # single fused fp8 convert (DVE); rT8 is the DoubleRow stationary
            rT8 = work.tile([128, 2, T, 128], F8, tag="rT8", name="rT8", bufs=4)
            cast = nc.vector.tensor_copy(rT8[:, :, :, :], rTb[:, :, :, :])
            return rT8, first, cast

        def emit_mm2(rT8, bsl, nchunks=((0, 512), (512, C - 512))):
            osb = work.tile([128, C], F32, tag="osb", name="osb")
            for n0, nsz in nchunks:
                ops = opool.tile([128, 512], F32, tag="ops", name="ops")
                for t_ in range(T):
                    nc.tensor.matmul(
                        ops[:, 0:nsz],
                        rT8[:, :, t_, :],
                        p_all[:, t_, :, n0 : n0 + nsz],
                        start=(t_ == 0),
                        stop=(t_ == T - 1),
                        perf_mode=DR,
                    )
                # descale on ACT (idle once the sigmoids are done)
                nc.scalar.mul(osb[:, n0 : n0 + nsz], ops[:, 0:nsz], ALPHA)
                nc.sync.dma_start(out[bsl, n0 : n0 + nsz], osb[:, n0 : n0 + nsz])

        # ---- b-outer mm1: each chunk finishes all its trees as early as the
        # W stream allows, so its routing chain overlaps the remaining mm1
        # work and rT8(b0) is ready well before the PE drains mm1. ----
        rT8s = {}
        prev_cast = None
        for bi in range(BCH):
            for j in range(TP):
                emit_mm1(bi, j)
            rT8s[bi], first_op, cast_op = emit_routing(bi)
            if prev_cast is not None:
                # pin DVE to chain order; the scheduler otherwise runs later
                # chunks' routing ahead of an earlier chunk's pending cast
                add_dep_helper(
                    first_op.ins, prev_cast.ins, sync=False, reason="chain order"
                )
            prev_cast = cast_op
        # ---- mm2 per chunk; last chunk splits finer to shorten the tail ----
        for bi in range(BCH - 1):
            emit_mm2(rT8s[bi], bass.ts(bi, 128))
        emit_mm2(
            rT8s[BCH - 1],
            bass.ts(BCH - 1, 128),
            nchunks=((0, 512), (512, 256), (768, C - 768)),
        )

    nc.finalize()
    return nc


_CACHED_NC = None
_WARMED = False


def _get_nc() -> bass.Bass:
    global _CACHED_NC
    if _CACHED_NC is None:
        _CACHED_NC = build_program()
    return _CACHED_NC


def _prep_inputs(l_input, cnn_w, final_probabilities):
    e4 = ml_dtypes.float8_e4m3fn

    def q8(a):
        # TRN e4m3 max normal is +-240 (OCP 256..448 are NaN on TRN)
        return np.clip(a, -240.0, 240.0).astype(e4)

    x = np.asarray(l_input, dtype=np.float32)
    W = np.asarray(cnn_w, dtype=np.float64)[:, :, NODE_PERM] * 64.0
    P = np.asarray(final_probabilities, dtype=np.float64)

    # x [B, F] -> [KC, 2, 128, B] -> [KC, 128, 2, B] fp8
    xT8 = np.ascontiguousarray(
        x.T.reshape(KC, 2, 128, B).transpose(0, 2, 1, 3)
    )
    xT8 = q8(xT8)

    # W [T, F, 255] -> pad nodes to 256 -> cols = (tree-pair local, node)
    Wp = np.zeros((T, F, NP), dtype=np.float64)
    Wp[:, :, :NODES] = W
    # -> [F, TP, 2*NP] -> [KC, 2, 128, TP, 2*NP] -> [TP, KC, 128, 2, 2*NP]
    Wr = (
        Wp.transpose(1, 0, 2)
        .reshape(F, TP, 2 * NP)
        .reshape(KC, 2, 128, TP, 2 * NP)
        .transpose(3, 0, 2, 1, 4)
    )
    Wr = q8(np.ascontiguousarray(Wr))

    # P: zero-mean over leaves; exact bias added on host
    Bm = P.mean(axis=1)                      # [T, C]
    bias_vec = (Bm.sum(axis=0) / T).astype(np.float32)   # [C]
    Pt = (P - Bm[:, None, :])[:, LEAF_PERM, :] * SP      # [T, 256, C] scaled
    # error-feedback quantization along the stored leaf order
    Pq = np.empty((T, LEAFS, C), dtype=e4)
    for t_ in range(T):
        carry = np.zeros(C, dtype=np.float64)
        for leaf in range(LEAFS):
            tgt = Pt[t_, leaf] + carry
            got = q8(tgt)
            Pq[t_, leaf] = got
            carry = tgt - got.astype(np.float64)
    # [T, 256, C] -> pad C to 1024 -> [T, 2, 128, CP] -> [128, T, 2, CP]
    Pp = np.zeros((T, 2, 128, CP), dtype=e4)
    Pp[:, :, :, :C] = Pq.reshape(T, 2, 128, C)
    Pr = np.ascontiguousarray(Pp.transpose(2, 0, 1, 3))
    return xT8, Wr, Pr, bias_vec


def _run(inputs, trace=False, trace_cores=None):
    xT8, Wr, Pr, bias_vec = _prep_inputs(
        inputs["l_input"], inputs["cnn_w"], inputs["final_probabilities"]
    )
    in_maps = [
        {
            "xT": np.ascontiguousarray(xT8[:, :, :, c * B_LOC : (c + 1) * B_LOC]),
            "w": Wr,
            "p": Pr,
        }
        for c in range(N_CORES)
    ]
    global _WARMED
    if not _WARMED and not trace:
        # one discarded execution to warm the device path (DMA rings, NEFF
        # residency, clock state) so the measured run is at steady state
        try:
            run_bass_kernel_spmd(
                _get_nc(), in_maps, core_ids=list(range(N_CORES)), trace=False
            )
        except Exception:
            pass
        _WARMED = True
    last_err = None
    for attempt in range(3):
        try:
            res = run_bass_kernel_spmd(
                _get_nc(),
                in_maps,
                core_ids=list(range(N_CORES)),
                trace=trace,
                trace_cores=trace_cores,
            )
            break
        except Exception as e:  # transient NRT device errors: retry
            last_err = e
            if attempt == 2:
                raise
            import time as _time

            _time.sleep(5)
    dev = np.concatenate([res.results[c]["out"] for c in range(N_CORES)], axis=0)
    out = np.clip(dev + bias_vec[None, :], 0.0, 1.0).astype(np.float32)
    return out, res


def kernel(**inputs) -> np.ndarray:
    out, _ = _run(inputs)
    return out


# revision 15
# speedup vs baseline: 1.1289x; 1.1289x over previous
"""Differentiable random-forest layer (inference path) on 8 Trainium2 cores.

Computation (per reference):
    d     = sigmoid(einsum('bf,tfn->btn', x, W))        # [B, T, 255]
    route = prod_l where(IS_LEFT, d[..n..], 1-d[..n..]) # [B, T, 256]
    out   = clip(einsum('btl,tlc->bc', route, P) / T, 0, 1)

Shapes: B=4096, F=1024, T=10 trees, 255 nodes / 256 leaves, C=1000.
Sharding: data-parallel over batch; 512 rows/core; no collectives.

v3: both matmuls in fp8(e4m3) with perf_mode=DoubleRow (256-row contraction
per matmul; measured 216ns per 512-col DR matmul at full p-state = 2x bf16).
The PE p-state drops ~2x after any idle gap and takes ~3us to recover, so
the schedule keeps the PE gapless from warmup to the last matmul:
  - mm1 groups emitted along (b+j) anti-diagonals: starts as soon as W j0
    lands while later W blocks stream in, and each batch chunk's sigmoid/
    routing chain starts as early as the DMA bandwidth allows.
  - routing runs split across DVE (most trees) and GpSimd (rest) per chunk;
    the bf16->fp8 casts of the transposed route run on GpSimd; sigmoids and
    the output descale-copies run on ACT. Chains pipeline under the PE.

Accuracy design (gate: rel err < 2e-2; measured 1.10e-2 for v2):
  - mm1: x fp8, W x64 in fp8; 1/64 folded into the sigmoid input scale.
  - routing in bf16 with the complement trick hi = cur - cur*d (no second
    sigmoid pass); route carries a x256 scale introduced at layer 0.
  - mm2: route fp8 after transpose; P decomposed as P = Ptilde + leafmean:
    the device matmul uses zero-mean Ptilde (x2^17, error-feedback
    quantized along leaves); the exact bias sum_t leafmean/T is added on
    the host. This kills the coherent route-error x mean(P) coupling.
  - reference clip(0,1): upper bound provably inactive; host clips.
"""

from contextlib import ExitStack

import numpy as np
import ml_dtypes

import concourse.bass as bass
import concourse.bacc as bacc
import concourse.mybir as mybir
import concourse.tile as tile
from concourse.tile import add_dep_helper
from concourse.bass_utils import run_bass_kernel_spmd

N_CORES = 8
B, F, T, NODES, LEAFS, C = 4096, 1024, 10, 255, 256, 1000
B_LOC = B // N_CORES            # 512 batch rows per core
BCH = B_LOC // 128              # 4 batch chunks of 128
KC = F // 256                   # 4 DoubleRow contraction chunks for mm1
TP = T // 2                     # 5 tree-pairs (2 trees -> 512 psum cols)
NP = 256                        # padded nodes per tree (255 + 1 pad col)
N_LAYERS = 8
CP = 1024                       # padded classes in SBUF

SP = float(2.0 ** 17)           # Ptilde fp8 scale
SR = 256.0                      # route scale (introduced at routing layer 0)
ALPHA = 1.0 / (SR * SP * T)     # psum2 -> output scale

WARM_N = 40                     # PE warmup matmuls (p-state ramp, ends ~when W j0 lands)

BF16 = mybir.dt.bfloat16
F8 = mybir.dt.float8e4
F32 = mybir.dt.float32
DR = mybir.MatmulPerfMode.DoubleRow
Sigmoid = mybir.ActivationFunctionType.Sigmoid
MULT = mybir.AluOpType.mult
ADD = mybir.AluOpType.add


def _bitrev(x: int, bits: int) -> int:
    r = 0
    for _ in range(bits):
        r = (r << 1) | (x & 1)
        x >>= 1
    return r


# Node-axis permutation: d'[.., off+q] = d[.., off+bitrev_l(q)] per layer l
NODE_PERM = np.empty(NODES, dtype=np.int64)
for _l in range(N_LAYERS):
    _off = (1 << _l) - 1
    for _q in range(1 << _l):
        NODE_PERM[_off + _q] = _off + _bitrev(_q, _l)
# Leaf-axis permutation: P'[t, q, :] = P[t, bitrev_8(q), :]
LEAF_PERM = np.array([_bitrev(q, N_LAYERS) for q in range(LEAFS)], dtype=np.int64)


def build_program() -> bass.Bass:
    nc = bacc.Bacc()

    xT = nc.dram_tensor("xT", [KC, 128, 2, B_LOC], F8, kind="ExternalInput")
    # W j-major: per tree-pair j: [128ki, KC, 2ko, 2*NP]
    w = nc.dram_tensor("w", [TP, KC, 128, 2, 2 * NP], F8, kind="ExternalInput")
    p = nc.dram_tensor("p", [128, T, 2, CP], F8, kind="ExternalInput")
    out = nc.dram_tensor("out", [B_LOC, C], F32, kind="ExternalOutput")

    with tile.TileContext(nc) as tc, ExitStack() as ctx:
        resident = ctx.enter_context(tc.tile_pool(name="resident", bufs=1))
        x_all = resident.tile([128, KC, 2, B_LOC], F8, tag="x_all", name="x_all")
        w_all = resident.tile([128, TP, KC, 2, 2 * NP], F8, tag="w_all", name="w_all")
        p_all = resident.tile([128, T, 2, CP], F8, tag="p_all", name="p_all")
        # Load order: x (every mm1 needs it), W j0 split fine so the first
        # matmul group can start earliest, the rest of W per (j, kc), then P
        # (mm2-only, arrives in time; keeping it last leaves the full HBM
        # bandwidth to the W stream that gates mm1).
        for kc in range(KC):
            nc.sync.dma_start(x_all[:, kc, :, :], xT[kc])
        for kc in range(KC):
            for ko in range(2):
                nc.sync.dma_start(w_all[:, 0, kc, ko, :], w[0, kc][:, ko])
        last_w = None
        for j in range(1, TP):
            for kc in range(KC):
                last_w = nc.sync.dma_start(w_all[:, j, kc, :, :], w[j, kc])
        # P is mm2-only: force it behind the W stream so W (which gates mm1)
        # gets the full HBM bandwidth. The DMA fabric otherwise round-robins
        # descriptors of every queued transfer, halving the W arrival rate.
        for t_ in range(0, T, 2):
            pd = nc.sync.dma_start(p_all[:, t_ : t_ + 2, :, :], p[:, t_ : t_ + 2, :, :])
            add_dep_helper(pd.ins, last_w.ins, sync=True, reason="P after W")

        dpool = ctx.enter_context(tc.tile_pool(name="dps", bufs=1, space="PSUM"))
        opool = ctx.enter_context(tc.tile_pool(name="ops", bufs=3, space="PSUM"))
        work = ctx.enter_context(tc.tile_pool(name="work", bufs=2))

        # PE warmup (p-state ramp) overlapping the input DMAs; also a dummy
        # activation so ACT's sigmoid table loads before the first real one.
        warm_in = work.tile([128, 128], BF16, tag="warm", name="warm_in", bufs=1)
        nc.vector.memset(warm_in[:, :], 0.0)
        warm_sig = work.tile([128, 1], BF16, tag="wsig", name="warm_sig", bufs=1)
        nc.scalar.activation(warm_sig[:, :], warm_in[:, 0:1], Sigmoid)
        warm_ps = opool.tile([128, 128], F32, tag="warm", name="warm_ps", bufs=1)
        for _ in range(WARM_N):
            nc.tensor.matmul(warm_ps[:, :], warm_in[:, :], warm_in[:, :])

        # d tiles: one per batch chunk, [128, tree, node(padded)] bf16
        ddb = [
            work.tile([128, T, NP], BF16, tag=f"ddb{b}", name=f"ddb{b}", bufs=1)
            for b in range(BCH)
        ]

        def emit_mm1(bi, j):
            # DoubleRow mm1 for (batch chunk bi, tree pair j) + sigmoid
            dps = dpool.tile([128, 2, NP], F32, tag="dps", name="dps", bufs=3)
            for kc in range(KC):
                nc.tensor.matmul(
                    dps[:, :, :],
                    x_all[:, kc, :, bass.ts(bi, 128)],
                    w_all[:, j, kc, :, :],
                    start=(kc == 0),
                    stop=(kc == KC - 1),
                    perf_mode=DR,
                )
            # d = sigmoid(logits/64) (W was scaled x64 on host)
            nc.scalar.activation(
                ddb[bi][:, 2 * j : 2 * j + 2, :], dps[:, :, :], Sigmoid, scale=1.0 / 64
            )

        def emit_routing(bi):
            # bf16 routing on DVE, complement trick, x256 scale.
            d = ddb[bi]
            Ra = work.tile([128, T, 128], BF16, tag="Ra", name="Ra")
            Rb = work.tile([128, T, 128], BF16, tag="Rb", name="Rb")
            routeC = work.tile([128, 2, T, 128], BF16, tag="routeC", name="routeC")
            # layer 0: lo = 256*d0 ; hi = 256 - lo
            first = nc.vector.tensor_scalar_mul(Ra[:, :, 0:1], d[:, :, 0:1], SR)
            nc.vector.tensor_scalar(Ra[:, :, 1:2], d[:, :, 0:1], -SR, SR, MULT, ADD)
            cur, nxt = Ra, Rb
            for l in range(1, N_LAYERS):
                w_l = 1 << l          # prefixes at layer l
                off = w_l - 1         # first node index of layer l
                if l < N_LAYERS - 1:
                    lo, hi = nxt[:, :, 0:w_l], nxt[:, :, w_l : 2 * w_l]
                else:
                    lo, hi = routeC[:, 0, :, :], routeC[:, 1, :, :]
                nc.vector.tensor_mul(lo, cur[:, :, 0:w_l], d[:, :, off : off + w_l])
                nc.vector.tensor_sub(hi, cur[:, :, 0:w_l], lo)
                cur, nxt = nxt, cur
            # transpose [b, leaf] -> [leaf, b] per leaf chunk (bf16 XBAR)
            rTb = work.tile([128, 2, T, 128], BF16, tag="rTb", name="rTb", bufs=2)
            nc.sync.dma_start_transpose(rTb[:, 0, :, :], routeC[:, 0])
            nc.sync.dma_start_transpose(rTb[:, 1, :, :], routeC[:, 1])
            # single fused fp8 convert (DVE); rT8 is the DoubleRow stationary
            rT8 = work.tile([128, 2, T, 128], F8, tag="rT8", name="rT8", bufs=4)
            cast = nc.vector.tensor_copy(rT8[:, :, :, :], rTb[:, :, :, :])
            return rT8, first, cast

        def emit_mm2(rT8, bsl, nchunks=((0, 512), (512, C - 512))):
            osb = work.tile([128, C], F32, tag="osb", name="osb")
            for n0, nsz in nchunks:
                ops = opool.tile([128, 512], F32, tag="ops", name="ops")
                for t_ in range(T):
                    nc.tensor.matmul(
                        ops[:, 0:nsz],
                        rT8[:, :, t_, :],
                        p_all[:, t_, :, n0 : n0 + nsz],
                        start=(t_ == 0),
                        stop=(t_ == T - 1),
                        perf_mode=DR,
                    )
                # descale on ACT (idle once the sigmoids are done)
                nc.scalar.mul(osb[:, n0 : n0 + nsz], ops[:, 0:nsz], ALPHA)
                nc.sync.dma_start(out[bsl, n0 : n0 + nsz], osb[:, n0 : n0 + nsz])

        # ---- mm1 order: b0/b1 interleaved while the W stream lands (the PE
        # outruns the per-block W arrival 2:1, so two chunks per block keep
        # it gapless), then b2/b3. Each chunk's routing chain is emitted as
        # soon as its last tree pair is done, overlapping the rest of mm1.
        MM1_ORDER = [
            (0, 0), (1, 0), (0, 1), (1, 1), (0, 2), (1, 2), (0, 3), (1, 3),
            (0, 4), (2, 0), (1, 4), (2, 1), (3, 0), (2, 2), (3, 1), (2, 3),
            (3, 2), (2, 4), (3, 3), (3, 4),
        ]
        rT8s = {}
        prev_cast = None
        for bi, j in MM1_ORDER:
            emit_mm1(bi, j)
            if j == TP - 1:
                rT8s[bi], first_op, cast_op = emit_routing(bi)
                if prev_cast is not None:
                    # pin DVE to chain order; the scheduler otherwise runs
                    # later chunks' routing ahead of a pending earlier cast
                    add_dep_helper(
                        first_op.ins, prev_cast.ins, sync=False, reason="chain order"
                    )
                prev_cast = cast_op
        # ---- mm2 per chunk; last chunk splits finer to shorten the tail ----
        for bi in range(BCH - 1):
            emit_mm2(rT8s[bi], bass.ts(bi, 128))
        emit_mm2(
            rT8s[BCH - 1],
            bass.ts(BCH - 1, 128),
            nchunks=((0, 512), (512, 256), (768, C - 768)),
        )

    nc.finalize()
    return nc


_CACHED_NC = None
_WARMED = False


def _get_nc() -> bass.Bass:
    global _CACHED_NC
    if _CACHED_NC is None:
        _CACHED_NC = build_program()
    return _CACHED_NC


def _prep_inputs(l_input, cnn_w, final_probabilities):
    e4 = ml_dtypes.float8_e4m3fn

    def q8(a):
        # TRN e4m3 max normal is +-240 (OCP 256..448 are NaN on TRN)
        return np.clip(a, -240.0, 240.0).astype(e4)

    x = np.asarray(l_input, dtype=np.float32)
    W = np.asarray(cnn_w, dtype=np.float64)[:, :, NODE_PERM] * 64.0
    P = np.asarray(final_probabilities, dtype=np.float64)

    # x [B, F] -> [KC, 2, 128, B] -> [KC, 128, 2, B] fp8
    xT8 = np.ascontiguousarray(
        x.T.reshape(KC, 2, 128, B).transpose(0, 2, 1, 3)
    )
    xT8 = q8(xT8)

    # W [T, F, 255] -> pad nodes to 256 -> cols = (tree-pair local, node)
    Wp = np.zeros((T, F, NP), dtype=np.float64)
    Wp[:, :, :NODES] = W
    # -> [F, TP, 2*NP] -> [KC, 2, 128, TP, 2*NP] -> [TP, KC, 128, 2, 2*NP]
    Wr = (
        Wp.transpose(1, 0, 2)
        .reshape(F, TP, 2 * NP)
        .reshape(KC, 2, 128, TP, 2 * NP)
        .transpose(3, 0, 2, 1, 4)
    )
    Wr = q8(np.ascontiguousarray(Wr))

    # P: zero-mean over leaves; exact bias added on host
    Bm = P.mean(axis=1)                      # [T, C]
    bias_vec = (Bm.sum(axis=0) / T).astype(np.float32)   # [C]
    Pt = (P - Bm[:, None, :])[:, LEAF_PERM, :] * SP      # [T, 256, C] scaled
    # error-feedback quantization along the stored leaf order
    Pq = np.empty((T, LEAFS, C), dtype=e4)
    for t_ in range(T):
        carry = np.zeros(C, dtype=np.float64)
        for leaf in range(LEAFS):
            tgt = Pt[t_, leaf] + carry
            got = q8(tgt)
            Pq[t_, leaf] = got
            carry = tgt - got.astype(np.float64)
    # [T, 256, C] -> pad C to 1024 -> [T, 2, 128, CP] -> [128, T, 2, CP]
    Pp = np.zeros((T, 2, 128, CP), dtype=e4)
    Pp[:, :, :, :C] = Pq.reshape(T, 2, 128, C)
    Pr = np.ascontiguousarray(Pp.transpose(2, 0, 1, 3))
    return xT8, Wr, Pr, bias_vec


def _run(inputs, trace=False, trace_cores=None):
    xT8, Wr, Pr, bias_vec = _prep_inputs(
        inputs["l_input"], inputs["cnn_w"], inputs["final_probabilities"]
    )
    in_maps = [
        {
            "xT": np.ascontiguousarray(xT8[:, :, :, c * B_LOC : (c + 1) * B_LOC]),
            "w": Wr,
            "p": Pr,
        }
        for c in range(N_CORES)
    ]
    global _WARMED
    if not _WARMED and not trace:
        # one discarded execution to warm the device path (DMA rings, NEFF
        # residency, clock state) so the measured run is at steady state
        try:
            run_bass_kernel_spmd(
                _get_nc(), in_maps, core_ids=list(range(N_CORES)), trace=False
            )
        except Exception:
            pass
        _WARMED = True
    last_err = None
    for attempt in range(3):
        try:
            res = run_bass_kernel_spmd(
                _get_nc(),
                in_maps,
                core_ids=list(range(N_CORES)),
                trace=trace,
                trace_cores=trace_cores,
            )
            break
        except Exception as e:  # transient NRT device errors: retry
            last_err = e
            if attempt == 2:
                raise
            import time as _time

            _time.sleep(5)
    dev = np.concatenate([res.results[c]["out"] for c in range(N_CORES)], axis=0)
    out = np.clip(dev + bias_vec[None, :], 0.0, 1.0).astype(np.float32)
    return out, res


def kernel(**inputs) -> np.ndarray:
    out, _ = _run(inputs)
    return out


# revision 18
# speedup vs baseline: 1.3696x; 1.2132x over previous
"""Differentiable random-forest layer (inference path) on 8 Trainium2 cores.

Computation (per reference):
    d     = sigmoid(einsum('bf,tfn->btn', x, W))        # [B, T, 255]
    route = prod_l where(IS_LEFT, d[..n..], 1-d[..n..]) # [B, T, 256]
    out   = clip(einsum('btl,tlc->bc', route, P) / T, 0, 1)

Shapes: B=4096, F=1024, T=10 trees, 255 nodes / 256 leaves, C=1000.
Sharding: data-parallel over batch; 512 rows/core; no collectives.

v3: both matmuls in fp8(e4m3) with perf_mode=DoubleRow (256-row contraction
per matmul; measured 216ns per 512-col DR matmul at full p-state = 2x bf16).
The PE p-state drops ~2x after any idle gap and takes ~3us to recover, so
the schedule keeps the PE gapless from warmup to the last matmul:
  - mm1 groups emitted along (b+j) anti-diagonals: starts as soon as W j0
    lands while later W blocks stream in, and each batch chunk's sigmoid/
    routing chain starts as early as the DMA bandwidth allows.
  - routing runs split across DVE (most trees) and GpSimd (rest) per chunk;
    the bf16->fp8 casts of the transposed route run on GpSimd; sigmoids and
    the output descale-copies run on ACT. Chains pipeline under the PE.

Accuracy design (gate: rel err < 2e-2; measured 1.10e-2 for v2):
  - mm1: x fp8, W x64 in fp8; 1/64 folded into the sigmoid input scale.
  - routing in bf16 with the complement trick hi = cur - cur*d (no second
    sigmoid pass); route carries a x256 scale introduced at layer 0.
  - mm2: route fp8 after transpose; P decomposed as P = Ptilde + leafmean:
    the device matmul uses zero-mean Ptilde (x2^17, error-feedback
    quantized along leaves); the exact bias sum_t leafmean/T is added on
    the host. This kills the coherent route-error x mean(P) coupling.
  - reference clip(0,1): upper bound provably inactive; host clips.
"""

from contextlib import ExitStack

import numpy as np
import ml_dtypes

import concourse.bass as bass
import concourse.bacc as bacc
import concourse.mybir as mybir
import concourse.tile as tile
from concourse.tile import add_dep_helper
from concourse.bass_utils import run_bass_kernel_spmd

N_CORES = 8
B, F, T, NODES, LEAFS, C = 4096, 1024, 10, 255, 256, 1000
B_LOC = B // N_CORES            # 512 batch rows per core
BCH = B_LOC // 128              # 4 batch chunks of 128
KC = F // 256                   # 4 DoubleRow contraction chunks for mm1
TP = T // 2                     # 5 tree-pairs (2 trees -> 512 psum cols)
NP = 256                        # padded nodes per tree (255 + 1 pad col)
N_LAYERS = 8
CP = 1024                       # padded classes in SBUF

SP = float(2.0 ** 17)           # Ptilde fp8 scale
SR = 256.0                      # route scale (introduced at routing layer 0)
ALPHA = 1.0 / (SR * SP * T)     # psum2 -> output scale

WARM_N = 40                     # PE warmup matmuls (p-state ramp, ends ~when W j0 lands)

BF16 = mybir.dt.bfloat16
F8 = mybir.dt.float8e4
F32 = mybir.dt.float32
DR = mybir.MatmulPerfMode.DoubleRow
Sigmoid = mybir.ActivationFunctionType.Sigmoid
MULT = mybir.AluOpType.mult
ADD = mybir.AluOpType.add


def _bitrev(x: int, bits: int) -> int:
    r = 0
    for _ in range(bits):
        r = (r << 1) | (x & 1)
        x >>= 1
    return r


# Node-axis permutation: d'[.., off+q] = d[.., off+bitrev_l(q)] per layer l
NODE_PERM = np.empty(NODES, dtype=np.int64)
for _l in range(N_LAYERS):
    _off = (1 << _l) - 1
    for _q in range(1 << _l):
        NODE_PERM[_off + _q] = _off + _bitrev(_q, _l)
# Leaf-axis permutation: P'[t, q, :] = P[t, bitrev_8(q), :]
LEAF_PERM = np.array([_bitrev(q, N_LAYERS) for q in range(LEAFS)], dtype=np.int64)


def build_program() -> bass.Bass:
    nc = bacc.Bacc()

    xT = nc.dram_tensor("xT", [KC, 128, 2, B_LOC], F8, kind="ExternalInput")
    # W j-major: per tree-pair j: [128ki, KC, 2ko, 2*NP]
    w = nc.dram_tensor("w", [TP, KC, 128, 2, 2 * NP], F8, kind="ExternalInput")
    p = nc.dram_tensor("p", [128, T, 2, CP], F8, kind="ExternalInput")
    out = nc.dram_tensor("out", [B_LOC, C], F32, kind="ExternalOutput")

    with tile.TileContext(nc) as tc, ExitStack() as ctx:
        resident = ctx.enter_context(tc.tile_pool(name="resident", bufs=1))
        x_all = resident.tile([128, KC, 2, B_LOC], F8, tag="x_all", name="x_all")
        w_all = resident.tile([128, TP, KC, 2, 2 * NP], F8, tag="w_all", name="w_all")
        p_all = resident.tile([128, T, 2, CP], F8, tag="p_all", name="p_all")
        # Load order: x (every mm1 needs it), W j0 split fine so the first
        # matmul group can start earliest, the rest of W per (j, kc), then P
        # (mm2-only, arrives in time; keeping it last leaves the full HBM
        # bandwidth to the W stream that gates mm1).
        for kc in range(KC):
            nc.sync.dma_start(x_all[:, kc, :, :], xT[kc])
        for kc in range(KC):
            for ko in range(2):
                nc.sync.dma_start(w_all[:, 0, kc, ko, :], w[0, kc][:, ko])
        last_w = None
        for j in range(1, TP):
            for kc in range(KC):
                last_w = nc.sync.dma_start(w_all[:, j, kc, :, :], w[j, kc])
        # P is mm2-only: force it behind the W stream so W (which gates mm1)
        # gets the full HBM bandwidth. The DMA fabric otherwise round-robins
        # descriptors of every queued transfer, halving the W arrival rate.
        for t_ in range(0, T, 2):
            pd = nc.sync.dma_start(p_all[:, t_ : t_ + 2, :, :], p[:, t_ : t_ + 2, :, :])
            add_dep_helper(pd.ins, last_w.ins, sync=True, reason="P after W")

        dpool = ctx.enter_context(tc.tile_pool(name="dps", bufs=1, space="PSUM"))
        opool = ctx.enter_context(tc.tile_pool(name="ops", bufs=3, space="PSUM"))
        work = ctx.enter_context(tc.tile_pool(name="work", bufs=2))

        # PE warmup (p-state ramp) overlapping the input DMAs; also a dummy
        # activation so ACT's sigmoid table loads before the first real one.
        warm_in = work.tile([128, 128], BF16, tag="warm", name="warm_in", bufs=1)
        nc.vector.memset(warm_in[:, :], 0.0)
        warm_sig = work.tile([128, 1], BF16, tag="wsig", name="warm_sig", bufs=1)
        nc.scalar.activation(warm_sig[:, :], warm_in[:, 0:1], Sigmoid)
        warm_ps = opool.tile([128, 128], F32, tag="warm", name="warm_ps", bufs=1)
        for _ in range(WARM_N):
            nc.tensor.matmul(warm_ps[:, :], warm_in[:, :], warm_in[:, :])

        # d tiles: one per batch chunk, [128, tree, node(padded)] bf16
        ddb = [
            work.tile([128, T, NP], BF16, tag=f"ddb{b}", name=f"ddb{b}", bufs=1)
            for b in range(BCH)
        ]

        def emit_mm1(bi, j):
            # DoubleRow mm1 for (batch chunk bi, tree pair j) + sigmoid
            dps = dpool.tile([128, 2, NP], F32, tag="dps", name="dps", bufs=3)
            for kc in range(KC):
                nc.tensor.matmul(
                    dps[:, :, :],
                    x_all[:, kc, :, bass.ts(bi, 128)],
                    w_all[:, j, kc, :, :],
                    start=(kc == 0),
                    stop=(kc == KC - 1),
                    perf_mode=DR,
                )
            # d = sigmoid(logits/64) (W was scaled x64 on host)
            nc.scalar.activation(
                ddb[bi][:, 2 * j : 2 * j + 2, :], dps[:, :, :], Sigmoid, scale=1.0 / 64
            )

        def emit_routing(bi):
            # bf16 routing on DVE, complement trick, x256 scale.
            d = ddb[bi]
            Ra = work.tile([128, T, 128], BF16, tag="Ra", name="Ra")
            Rb = work.tile([128, T, 128], BF16, tag="Rb", name="Rb")
            routeC = work.tile([128, 2, T, 128], BF16, tag="routeC", name="routeC")
            # layer 0: lo = 256*d0 ; hi = 256 - lo
            nc.vector.tensor_scalar_mul(Ra[:, :, 0:1], d[:, :, 0:1], SR)
            nc.vector.tensor_scalar(Ra[:, :, 1:2], d[:, :, 0:1], -SR, SR, MULT, ADD)
            cur, nxt = Ra, Rb
            for l in range(1, N_LAYERS):
                w_l = 1 << l          # prefixes at layer l
                off = w_l - 1         # first node index of layer l
                if l < N_LAYERS - 1:
                    lo, hi = nxt[:, :, 0:w_l], nxt[:, :, w_l : 2 * w_l]
                else:
                    lo, hi = routeC[:, 0, :, :], routeC[:, 1, :, :]
                nc.vector.tensor_mul(lo, cur[:, :, 0:w_l], d[:, :, off : off + w_l])
                nc.vector.tensor_sub(hi, cur[:, :, 0:w_l], lo)
                cur, nxt = nxt, cur
            # transpose [b, leaf] -> [leaf, b] per leaf chunk (bf16 XBAR)
            rTb = work.tile([128, 2, T, 128], BF16, tag="rTb", name="rTb", bufs=2)
            nc.sync.dma_start_transpose(rTb[:, 0, :, :], routeC[:, 0])
            nc.sync.dma_start_transpose(rTb[:, 1, :, :], routeC[:, 1])
            # fp8 convert via a gpsimd-initiated casting DMA (bit-exact RNE,
            # measured ~0.7us trigger + ~1us fabric) - costs no DVE/ACT time
            rT8 = work.tile([128, 2, T, 128], F8, tag="rT8", name="rT8", bufs=4)
            nc.gpsimd.dma_start(rT8[:, :, :, :], rTb[:, :, :, :])
            return rT8

        def emit_mm2(rT8, bsl, nchunks=((0, 512), (512, C - 512))):
            osb = work.tile([128, C], F32, tag="osb", name="osb")
            for n0, nsz in nchunks:
                ops = opool.tile([128, 512], F32, tag="ops", name="ops")
                for t_ in range(T):
                    nc.tensor.matmul(
                        ops[:, 0:nsz],
                        rT8[:, :, t_, :],
                        p_all[:, t_, :, n0 : n0 + nsz],
                        start=(t_ == 0),
                        stop=(t_ == T - 1),
                        perf_mode=DR,
                    )
                # descale on ACT (idle once the sigmoids are done)
                nc.scalar.mul(osb[:, n0 : n0 + nsz], ops[:, 0:nsz], ALPHA)
                nc.sync.dma_start(out[bsl, n0 : n0 + nsz], osb[:, n0 : n0 + nsz])

        # ---- mm1 order: b0/b1 interleaved while the W stream lands (the PE
        # outruns the per-block W arrival 2:1, so two chunks per block keep
        # it gapless), then b2/b3. Each chunk's routing chain is emitted as
        # soon as its last tree pair is done, overlapping the rest of mm1.
        MM1_ORDER = [
            (0, 0), (1, 0), (0, 1), (1, 1), (0, 2), (1, 2), (0, 3), (1, 3),
            (0, 4), (2, 0), (1, 4), (2, 1), (3, 0), (2, 2), (3, 1), (2, 3),
            (3, 2), (2, 4), (3, 3), (3, 4),
        ]
        rT8s = {}
        for bi, j in MM1_ORDER:
            emit_mm1(bi, j)
            if j == TP - 1:
                rT8s[bi] = emit_routing(bi)
        # ---- mm2 per chunk; last chunk splits finer to shorten the tail ----
        for bi in range(BCH - 1):
            emit_mm2(rT8s[bi], bass.ts(bi, 128))
        emit_mm2(
            rT8s[BCH - 1],
            bass.ts(BCH - 1, 128),
            nchunks=((0, 512), (512, 256), (768, C - 768)),
        )

    nc.finalize()
    return nc


_CACHED_NC = None
_WARMED = False


def _get_nc() -> bass.Bass:
    global _CACHED_NC
    if _CACHED_NC is None:
        _CACHED_NC = build_program()
    return _CACHED_NC


def _prep_inputs(l_input, cnn_w, final_probabilities):
    e4 = ml_dtypes.float8_e4m3fn

    def q8(a):
        # TRN e4m3 max normal is +-240 (OCP 256..448 are NaN on TRN)
        return np.clip(a, -240.0, 240.0).astype(e4)

    x = np.asarray(l_input, dtype=np.float32)
    W = np.asarray(cnn_w, dtype=np.float64)[:, :, NODE_PERM] * 64.0
    P = np.asarray(final_probabilities, dtype=np.float64)

    # x [B, F] -> [KC, 2, 128, B] -> [KC, 128, 2, B] fp8
    xT8 = np.ascontiguousarray(
        x.T.reshape(KC, 2, 128, B).transpose(0, 2, 1, 3)
    )
    xT8 = q8(xT8)

    # W [T, F, 255] -> pad nodes to 256 -> cols = (tree-pair local, node)
    Wp = np.zeros((T, F, NP), dtype=np.float64)
    Wp[:, :, :NODES] = W
    # -> [F, TP, 2*NP] -> [KC, 2, 128, TP, 2*NP] -> [TP, KC, 128, 2, 2*NP]
    Wr = (
        Wp.transpose(1, 0, 2)
        .reshape(F, TP, 2 * NP)
        .reshape(KC, 2, 128, TP, 2 * NP)
        .transpose(3, 0, 2, 1, 4)
    )
    Wr = q8(np.ascontiguousarray(Wr))

    # P: zero-mean over leaves; exact bias added on host
    Bm = P.mean(axis=1)                      # [T, C]
    bias_vec = (Bm.sum(axis=0) / T).astype(np.float32)   # [C]
    Pt = (P - Bm[:, None, :])[:, LEAF_PERM, :] * SP      # [T, 256, C] scaled
    # error-feedback quantization along the stored leaf order
    Pq = np.empty((T, LEAFS, C), dtype=e4)
    for t_ in range(T):
        carry = np.zeros(C, dtype=np.float64)
        for leaf in range(LEAFS):
            tgt = Pt[t_, leaf] + carry
            got = q8(tgt)
            Pq[t_, leaf] = got
            carry = tgt - got.astype(np.float64)
    # [T, 256, C] -> pad C to 1024 -> [T, 2, 128, CP] -> [128, T, 2, CP]
    Pp = np.zeros((T, 2, 128, CP), dtype=e4)
    Pp[:, :, :, :C] = Pq.reshape(T, 2, 128, C)
    Pr = np.ascontiguousarray(Pp.transpose(2, 0, 1, 3))
    return xT8, Wr, Pr, bias_vec


def _run(inputs, trace=False, trace_cores=None):
    xT8, Wr, Pr, bias_vec = _prep_inputs(
        inputs["l_input"], inputs["cnn_w"], inputs["final_probabilities"]
    )
    in_maps = [
        {
            "xT": np.ascontiguousarray(xT8[:, :, :, c * B_LOC : (c + 1) * B_LOC]),
            "w": Wr,
            "p": Pr,
        }
        for c in range(N_CORES)
    ]
    global _WARMED
    if not _WARMED and not trace:
        # one discarded execution to warm the device path (DMA rings, NEFF
        # residency, clock state) so the measured run is at steady state
        try:
            run_bass_kernel_spmd(
                _get_nc(), in_maps, core_ids=list(range(N_CORES)), trace=False
            )
        except Exception:
            pass
        _WARMED = True
    last_err = None
    for attempt in range(3):
        try:
            res = run_bass_kernel_spmd(
                _get_nc(),
                in_maps,
                core_ids=list(range(N_CORES)),
                trace=trace,
                trace_cores=trace_cores,
            )
            break
        except Exception as e:  # transient NRT device errors: retry
            last_err = e
            if attempt == 2:
                raise
            import time as _time

            _time.sleep(5)
    dev = np.concatenate([res.results[c]["out"] for c in range(N_CORES)], axis=0)
    out = np.clip(dev + bias_vec[None, :], 0.0, 1.0).astype(np.float32)
    return out, res


def kernel(**inputs) -> np.ndarray:
    out, _ = _run(inputs)
    return out
